# revision 4
# baseline (speedup 1.0000x reference)
import sys
sys.path.insert(0, '/opt/trn_rl_repo')
import numpy as np
import math

import concourse.bass as bass
import concourse.mybir as mybir
import concourse.tile as tile
from concourse import bacc
from concourse.bass_utils import run_bass_kernel_spmd

# Problem dims
B, SL, CH, HZ = 128, 5000, 12, 100
L, D, DFF, H, NCLS = 5, 1024, 4096, 16, 71
NI = CH * HZ          # 1200
S = SL // HZ          # 50
NCORES = 8
NB = B // NCORES      # 16 batches per core
T = NB * S            # 800 tokens per core
NIP = 1280            # padded input-feature dim (1200 + 50 one-hot -> 1250 -> 1280)
NKI = NIP // 128      # 10 input k-chunks
DK = D // H           # 64
NDC = D // 128        # 8 d-chunks
NFC = DFF // 128      # 32 ff-chunks
HT = T // 2           # 400 tokens per half (psum moving-dim limit)

F32R = mybir.dt.float32r
F32 = mybir.dt.float32
BF16 = mybir.dt.bfloat16
EXP = mybir.ActivationFunctionType.Exp
RELU = mybir.ActivationFunctionType.Relu
SQUARE = mybir.ActivationFunctionType.Square
IDENT = mybir.ActivationFunctionType.Identity
COPY = mybir.ActivationFunctionType.Copy
AOP = mybir.AluOpType

TRACE = False
LAST_EXEC_NS = None
_CACHE = {}

# bias-pack columns in bL
BQ, BK, B1, BO, B2 = 0, 8, 16, 48, 56


def _build(n_layers=L):
    nc = bacc.Bacc(None)
    d = {}
    d['xT'] = nc.dram_tensor("xT", [128, NKI * T], BF16, kind="ExternalInput")
    d['ones'] = nc.dram_tensor("ones", [128, 512], F32R, kind="ExternalInput")
    d['emb_w'] = nc.dram_tensor("emb_w", [2, 128, 4, NKI, 128], BF16, kind="ExternalInput")
    d['qk_w'] = nc.dram_tensor("qk_w", [L, 2, 128, NDC, NDC, 128], BF16, kind="ExternalInput")
    d['wv_w'] = nc.dram_tensor("wv_w", [L, 128, NDC, D], BF16, kind="ExternalInput")
    d['wo_w'] = nc.dram_tensor("wo_w", [L, 128, NDC, NDC, 128], BF16, kind="ExternalInput")
    d['w1_w'] = nc.dram_tensor("w1_w", [L, 4, 128, 8, NDC, 128], BF16, kind="ExternalInput")
    d['w2_w'] = nc.dram_tensor("w2_w", [L, 4, 128, 2, NFC, 128], BF16, kind="ExternalInput")
    d['bL'] = nc.dram_tensor("bL", [L, 128, 64], F32, kind="ExternalInput")
    d['cf_w'] = nc.dram_tensor("cf_w", [128, NDC, NDC, 128], BF16, kind="ExternalInput")
    d['cf_bT'] = nc.dram_tensor("cf_bT", [128, NDC], F32, kind="ExternalInput")
    d['fc_w'] = nc.dram_tensor("fc_w", [128, NDC, NCLS], BF16, kind="ExternalInput")
    d['fc_b'] = nc.dram_tensor("fc_b", [NCLS, 1], F32, kind="ExternalInput")
    out = nc.dram_tensor("out", [NCLS, NB], F32, kind="ExternalOutput")

    with tile.TileContext(nc) as tc:
        _emit(nc, tc, d, out, n_layers)
    nc.compile()
    return nc


def _emit(nc, tc, d, out, n_layers):
    import contextlib
    ctx = contextlib.ExitStack()
    with ctx:
        sb1 = ctx.enter_context(tc.tile_pool(name="sb1", bufs=1))
        wp = ctx.enter_context(tc.tile_pool(name="wp", bufs=3))
        sq_p = ctx.enter_context(tc.tile_pool(name="sqp", bufs=4))
        rows = ctx.enter_context(tc.tile_pool(name="rows", bufs=4))
        rden_p = ctx.enter_context(tc.tile_pool(name="rden", bufs=4))
        brow_p = ctx.enter_context(tc.tile_pool(name="brow", bufs=2))
        pt_p = ctx.enter_context(tc.tile_pool(name="ptp", bufs=2))
        bc_p = ctx.enter_context(tc.tile_pool(name="bcp", bufs=2))
        osc_p = ctx.enter_context(tc.tile_pool(name="oscp", bufs=2))
        ps_mm = ctx.enter_context(tc.tile_pool(name="psmm", bufs=3, space="PSUM"))
        ps_at = ctx.enter_context(tc.tile_pool(name="psat", bufs=3, space="PSUM"))
        ps_row = ctx.enter_context(tc.tile_pool(name="psrow", bufs=2, space="PSUM"))

        # persistent tiles
        hT = sb1.tile([128, NDC, T], F32R, tag="hT")
        aPad = sb1.tile([128, NDC, NB * 64], BF16, tag="aPad")
        ones_c = sb1.tile([128, 1], F32R, tag="ones_c")
        ones_r = sb1.tile([1, 512], F32R, tag="ones_r")
        ones_cb = sb1.tile([128, 1], BF16, tag="ones_cb")
        ones_rb = sb1.tile([1, 512], BF16, tag="ones_rb")
        nc.sync.dma_start(ones_c[:], d['ones'][:, 0:1])
        nc.sync.dma_start(ones_r[:], d['ones'][0:1, :])
        nc.vector.tensor_copy(ones_cb[:], ones_c[:])
        nc.vector.tensor_copy(ones_rb[:], ones_r[:])

        def ln_half(src, col0, ncols, dst, dcol0, nch=NDC):
            """Plain LN over feature dim (nch*128) of src[:, :, col0:col0+ncols]
            -> dst[:, :, dcol0:dcol0+ncols] (bf16). src is [128, nch, *] f32r."""
            Dn = float(nch * 128)
            cm = 1.0 / Dn
            cv2 = 1.0 / (Dn - 1.0)
            cv1 = -1.0 / (Dn * (Dn - 1.0))
            s1 = ps_row.tile([1, ncols], F32, tag="row")
            s2 = ps_row.tile([1, ncols], F32, tag="row")
            for c in range(nch):
                sq = sq_p.tile([128, ncols], F32R, tag="sq")
                nc.scalar.square(sq[:], src[:, c, col0:col0 + ncols])
                nc.tensor.matmul(s1[:], ones_c[:], src[:, c, col0:col0 + ncols],
                                 start=(c == 0), stop=(c == nch - 1))
                nc.tensor.matmul(s2[:], ones_c[:], sq[:],
                                 start=(c == 0), stop=(c == nch - 1))
            m_row = rows.tile([1, ncols], F32R, tag="rowsb")
            t1 = rows.tile([1, ncols], F32, tag="rowsb")
            t2 = rows.tile([1, ncols], F32, tag="rowsb")
            nc.vector.tensor_scalar_mul(m_row[:], s1[:], cm)
            # var = s2*cv2 - (s1*sqrt(-cv1))^2 ; scale folded into the square
            nc.scalar.activation(t1[:], s1[:], SQUARE, bias=0.0, scale=math.sqrt(-cv1))
            nc.vector.tensor_scalar_mul(t2[:], s2[:], cv2)
            nc.vector.tensor_tensor(out=t2[:], in0=t2[:], in1=t1[:], op=AOP.subtract)
            nc.scalar.sqrt(t2[:], t2[:])
            nc.vector.tensor_scalar_add(t2[:], t2[:], 1e-6)
            r_row = rows.tile([1, ncols], F32R, tag="rowsb")
            with nc.allow_low_precision(reason="fp32r rounding of 1/(std+eps)"):
                nc.vector.reciprocal(r_row[:], t2[:])
            Mb = ps_at.tile([128, ncols], F32, tag="at")
            Rb = ps_at.tile([128, ncols], F32, tag="at")
            nc.tensor.matmul(Mb[:], ones_r[0:1, 0:128], m_row[:], start=True, stop=True)
            nc.tensor.matmul(Rb[:], ones_r[0:1, 0:128], r_row[:], start=True, stop=True)
            for c in range(nch):
                nc.vector.tensor_tensor(out=dst[:, c, dcol0:dcol0 + ncols],
                                        in0=src[:, c, col0:col0 + ncols],
                                        in1=Mb[:], op=AOP.subtract)
                nc.vector.tensor_tensor(out=dst[:, c, dcol0:dcol0 + ncols],
                                        in0=dst[:, c, dcol0:dcol0 + ncols],
                                        in1=Rb[:], op=AOP.mult)

        # ---------------- embed ----------------
        xt = sb1.tile([128, NKI, T], BF16, tag="tagV")
        for k in range(NKI):
            eng = nc.sync if k % 2 == 0 else nc.gpsimd
            eng.dma_start(xt[:, k, :], d['xT'][:, k * T:(k + 1) * T])
        for g in range(2):
            wt = wp.tile([128, 4, NKI, 128], BF16, tag="wp")
            for m4 in range(4):
                nc.sync.dma_start(wt[:, m4], d['emb_w'][g, :, m4])
            for hf in range(2):
                for m4 in range(4):
                    m = g * 4 + m4
                    ps = ps_mm.tile([128, HT], F32, tag="mm")
                    for k in range(NKI):
                        nc.tensor.matmul(ps[:], wt[:, m4, k, :], xt[:, k, hf * HT:(hf + 1) * HT],
                                         start=(k == 0), stop=(k == NKI - 1))
                    nc.scalar.activation(hT[:, m, hf * HT:(hf + 1) * HT], ps[:], COPY)

        # ---------------- layers ----------------
        hL = None
        for li in range(n_layers):
            last = (li == n_layers - 1) and (n_layers == L)
            # ---- LN1 -> aTb
            aTb = sb1.tile([128, NDC, T], BF16, tag="aT")
            for hf in range(2):
                ln_half(hT, hf * HT, HT, aTb, hf * HT)
            bL = brow_p.tile([128, 64], F32, tag="brow")
            nc.sync.dma_start(bL[:], d['bL'][li])
            # ---- Q, K  (last layer: K full, Q only for each batch's final token)
            qT = sb1.tile([128, NDC, T], BF16, tag="tagQ")
            kT = sb1.tile([128, NDC, T], BF16, tag="tagK")
            if last:
                aL16 = sb1.tile([128, NDC, NB], BF16, tag="aL16")
                for c in range(NDC):
                    nc.vector.tensor_copy(
                        aL16[:, c, :],
                        aTb[:, c, :].rearrange("p (b s) -> p b s", s=S)[:, :, S - 1])
                qL = sb1.tile([128, NDC, NB], BF16, tag="qL")
                wt = wp.tile([128, NDC, NDC, 128], BF16, tag="wp")
                nc.sync.dma_start(wt[:], d['qk_w'][li, 0])
                for m in range(NDC):
                    ps = ps_mm.tile([128, NB], F32, tag="mm")
                    for k in range(NDC):
                        nc.tensor.matmul(ps[:], wt[:, m, k, :], aL16[:, k, :],
                                         start=(k == 0), stop=(k == NDC - 1))
                    nc.vector.tensor_scalar_add(qL[:, m, :], ps[:], bL[:, m:m + 1])
                mats = ((1, kT),)
            else:
                mats = ((0, qT), (1, kT))
            for mat, dstT in mats:
                wt = wp.tile([128, NDC, NDC, 128], BF16, tag="wp")
                nc.sync.dma_start(wt[:], d['qk_w'][li, mat])
                for hf in range(2):
                    for m in range(NDC):
                        ps = ps_mm.tile([128, HT], F32, tag="mm")
                        for k in range(NDC):
                            nc.tensor.matmul(ps[:], wt[:, m, k, :],
                                             aTb[:, k, hf * HT:(hf + 1) * HT],
                                             start=(k == 0), stop=(k == NDC - 1))
                        nc.vector.tensor_scalar_add(
                            dstT[:, m, hf * HT:(hf + 1) * HT], ps[:],
                            bL[:, mat * NDC + m:mat * NDC + m + 1])
            # ---- V (token-major; batch pair per chunk at partition bases 0 / 64)
            # aPad: aTb re-laid-out with each 50-token batch padded to a 64-token
            # slot, so a 128-column slice = one batch pair at partition bases 0/64.
            if li == 0:
                for c in range(NDC):
                    pad = aPad[:, c, :].rearrange("p (b z) -> p b z", z=64)[:, :, S:64]
                    nc.vector.tensor_scalar_mul(
                        pad, aTb[:, c, 0:NB * (64 - S)].rearrange(
                            "p (b s) -> p b s", s=64 - S), 0.0)
            for c in range(NDC):
                nc.vector.tensor_copy(
                    aPad[:, c, :].rearrange("p (b z) -> p b z", z=64)[:, :, 0:S],
                    aTb[:, c, :].rearrange("p (b s) -> p b s", s=S))
            v = sb1.tile([128, NDC, D], BF16, tag="tagV")
            wv = wp.tile([128, NDC, D], BF16, tag="wp")
            nc.sync.dma_start(wv[:], d['wv_w'][li])
            for t4 in range(NDC):
                p0 = t4 * 128
                for n in range(2):
                    ps = ps_mm.tile([128, 512], F32, tag="mm")
                    for k in range(NDC):
                        nc.tensor.matmul(ps[:], aPad[:, k, p0:p0 + 128],
                                         wv[:, k, n * 512:(n + 1) * 512],
                                         start=(k == 0), stop=(k == NDC - 1))
                    nc.scalar.activation(v[:, t4, n * 512:(n + 1) * 512], ps[:], COPY)
            # ---- attention, one batch PAIR per block (partition bases 0 / 64)
            oT = sb1.tile([128, NDC, T], BF16, tag="tagO")
            osc = None
            for pi in ([] if last else range(NB // 2)):
                bc0 = pi * 2 * S
                psE = ps_at.tile([128, 8 * S], F32, tag="at")
                psO = ps_at.tile([128, 8 * S], F32, tag="at")
                pTE = pt_p.tile([128, 8 * S], BF16, tag="pt")
                pTO = pt_p.tile([128, 8 * S], BF16, tag="pt")
                for par in range(2):
                    vb = par * 64
                    b0 = bc0 + par * S
                    for c in range(NDC):
                        nc.tensor.matmul(psE[vb:vb + S, c * S:(c + 1) * S],
                                         kT[0:DK, c, b0:b0 + S], qT[0:DK, c, b0:b0 + S],
                                         start=True, stop=True)
                    for c in range(NDC):
                        nc.tensor.matmul(psO[vb:vb + S, c * S:(c + 1) * S],
                                         kT[DK:128, c, b0:b0 + S], qT[DK:128, c, b0:b0 + S],
                                         start=True, stop=True)
                    nc.scalar.activation(pTE[vb:vb + S], psE[vb:vb + S], EXP,
                                         bias=0.0, scale=1.0 / math.sqrt(DK))
                    nc.scalar.activation(pTO[vb:vb + S], psO[vb:vb + S], EXP,
                                         bias=0.0, scale=1.0 / math.sqrt(DK))
                # denominators land in dead rows of the score tiles (bases 0/64)
                rds = []
                for par in range(2):
                    vb = par * 64
                    nc.tensor.matmul(psE[vb:vb + 1, :], ones_cb[vb:vb + S, :],
                                     pTE[vb:vb + S], start=True, stop=True)
                    nc.tensor.matmul(psO[vb:vb + 1, :], ones_cb[vb:vb + S, :],
                                     pTO[vb:vb + S], start=True, stop=True)
                    rdE = rden_p.tile([1, 8 * S], BF16, tag="rden")
                    rdO = rden_p.tile([1, 8 * S], BF16, tag="rden")
                    with nc.allow_low_precision(reason="softmax denom reciprocal"):
                        nc.vector.reciprocal(rdE[:], psE[vb:vb + 1, :])
                        nc.vector.reciprocal(rdO[:], psO[vb:vb + 1, :])
                    rds.append((rdE, rdO))
                # PV on unnormalized probs; fold 1/den into the oT write
                poE = ps_mm.tile([128, 8 * S], F32, tag="mm")
                poO = ps_mm.tile([128, 8 * S], F32, tag="mm")
                bcE, bcO = psE, psO   # reuse dead score tiles for the broadcasts
                for par in range(2):
                    vb = par * 64
                    for c in range(NDC):
                        nc.tensor.matmul(poE[vb:vb + DK, c * S:(c + 1) * S],
                                         v[vb:vb + S, pi, (2 * c) * DK:(2 * c + 1) * DK],
                                         pTE[vb:vb + S, c * S:(c + 1) * S],
                                         start=True, stop=True)
                    for c in range(NDC):
                        nc.tensor.matmul(poO[vb:vb + DK, c * S:(c + 1) * S],
                                         v[vb:vb + S, pi, (2 * c + 1) * DK:(2 * c + 2) * DK],
                                         pTO[vb:vb + S, c * S:(c + 1) * S],
                                         start=True, stop=True)
                    nc.tensor.matmul(bcE[vb:vb + DK, :], ones_rb[0:1, 0:DK],
                                     rds[par][0][:], start=True, stop=True)
                    nc.tensor.matmul(bcO[vb:vb + DK, :], ones_rb[0:1, 0:DK],
                                     rds[par][1][:], start=True, stop=True)
                bcEs = bc_p.tile([128, 8 * S], BF16, tag="bc")
                bcOs = bc_p.tile([128, 8 * S], BF16, tag="bc")
                nc.scalar.activation(bcEs[:], bcE[:], COPY)
                nc.scalar.activation(bcOs[:], bcO[:], COPY)
                for par in range(2):
                    vb = par * 64
                    b0 = bc0 + par * S
                    bi = 2 * pi + par
                    # even heads -> oT rows 0-63 directly (normalize on the way out)
                    nc.vector.tensor_tensor(
                        out=oT[0:DK, :, b0:b0 + S],
                        in0=poE[vb:vb + DK].rearrange("p (c t) -> p c t", c=NDC),
                        in1=bcEs[vb:vb + DK].rearrange("p (c t) -> p c t", c=NDC),
                        op=AOP.mult)
                    # odd heads -> scratch; flush 8 batches per DMA into rows 64-127
                    if bi % 8 == 0:
                        osc = osc_p.tile([DK, NDC, 8, S], BF16, tag="osc")
                    nc.vector.tensor_tensor(
                        out=osc[:, :, bi % 8, :],
                        in0=poO[vb:vb + DK].rearrange("p (c t) -> p c t", c=NDC),
                        in1=bcOs[vb:vb + DK].rearrange("p (c t) -> p c t", c=NDC),
                        op=AOP.mult)
                    if bi % 8 == 7:
                        nc.sync.dma_start(
                            oT[DK:128, :, (bi - 7) * S:(bi + 1) * S],
                            osc[:].rearrange("p c b s -> p c (b s)"))
            if last:
                # last layer: attention + Wo only for each batch's final token
                oL = sb1.tile([128, NDC, NB], BF16, tag="oL")
                osc16 = osc_p.tile([DK, NDC, NB], BF16, tag="osc")
                for pi in range(NB // 2):
                    psE = ps_at.tile([128, NDC], F32, tag="at")
                    psO = ps_at.tile([128, NDC], F32, tag="at")
                    pTE = pt_p.tile([128, NDC], BF16, tag="pt")
                    pTO = pt_p.tile([128, NDC], BF16, tag="pt")
                    for par in range(2):
                        vb = par * 64
                        bi = 2 * pi + par
                        bc0 = bi * S
                        for c in range(NDC):
                            nc.tensor.matmul(psE[vb:vb + S, c:c + 1],
                                             kT[0:DK, c, bc0:bc0 + S], qL[0:DK, c, bi:bi + 1],
                                             start=True, stop=True)
                        for c in range(NDC):
                            nc.tensor.matmul(psO[vb:vb + S, c:c + 1],
                                             kT[DK:128, c, bc0:bc0 + S], qL[DK:128, c, bi:bi + 1],
                                             start=True, stop=True)
                        nc.scalar.activation(pTE[vb:vb + S], psE[vb:vb + S], EXP,
                                             bias=0.0, scale=1.0 / math.sqrt(DK))
                        nc.scalar.activation(pTO[vb:vb + S], psO[vb:vb + S], EXP,
                                             bias=0.0, scale=1.0 / math.sqrt(DK))
                    rds = []
                    for par in range(2):
                        vb = par * 64
                        nc.tensor.matmul(psE[vb:vb + 1, :], ones_cb[vb:vb + S, :],
                                         pTE[vb:vb + S], start=True, stop=True)
                        nc.tensor.matmul(psO[vb:vb + 1, :], ones_cb[vb:vb + S, :],
                                         pTO[vb:vb + S], start=True, stop=True)
                        rdE = rden_p.tile([1, NDC], BF16, tag="rden")
                        rdO = rden_p.tile([1, NDC], BF16, tag="rden")
                        with nc.allow_low_precision(reason="softmax denom reciprocal"):
                            nc.vector.reciprocal(rdE[:], psE[vb:vb + 1, :])
                            nc.vector.reciprocal(rdO[:], psO[vb:vb + 1, :])
                        rds.append((rdE, rdO))
                    poE = ps_mm.tile([128, NDC], F32, tag="mm")
                    poO = ps_mm.tile([128, NDC], F32, tag="mm")
                    bcE, bcO = psE, psO   # reuse dead score tiles for the broadcasts
                    for par in range(2):
                        vb = par * 64
                        for c in range(NDC):
                            nc.tensor.matmul(poE[vb:vb + DK, c:c + 1],
                                             v[vb:vb + S, pi, (2 * c) * DK:(2 * c + 1) * DK],
                                             pTE[vb:vb + S, c:c + 1], start=True, stop=True)
                        for c in range(NDC):
                            nc.tensor.matmul(poO[vb:vb + DK, c:c + 1],
                                             v[vb:vb + S, pi, (2 * c + 1) * DK:(2 * c + 2) * DK],
                                             pTO[vb:vb + S, c:c + 1], start=True, stop=True)
                        nc.tensor.matmul(bcE[vb:vb + DK, :], ones_rb[0:1, 0:DK],
                                         rds[par][0][:], start=True, stop=True)
                        nc.tensor.matmul(bcO[vb:vb + DK, :], ones_rb[0:1, 0:DK],
                                         rds[par][1][:], start=True, stop=True)
                    bcEs = bc_p.tile([128, NDC], BF16, tag="bc")
                    bcOs = bc_p.tile([128, NDC], BF16, tag="bc")
                    nc.scalar.activation(bcEs[:], bcE[:], COPY)
                    nc.scalar.activation(bcOs[:], bcO[:], COPY)
                    for par in range(2):
                        vb = par * 64
                        bi = 2 * pi + par
                        nc.vector.tensor_tensor(out=oL[0:DK, :, bi], in0=poE[vb:vb + DK],
                                                in1=bcEs[vb:vb + DK], op=AOP.mult)
                        nc.vector.tensor_tensor(out=osc16[:, :, bi], in0=poO[vb:vb + DK],
                                                in1=bcOs[vb:vb + DK], op=AOP.mult)
                nc.sync.dma_start(oL[DK:128, :, :], osc16[:])
                # small Wo + residual on last-token columns only
                hL = sb1.tile([128, NDC, NB], F32R, tag="hL")
                for c in range(NDC):
                    nc.vector.tensor_copy(
                        hL[:, c, :],
                        hT[:, c, :].rearrange("p (b s) -> p b s", s=S)[:, :, S - 1])
                wt = wp.tile([128, NDC, NDC, 128], BF16, tag="wp")
                nc.sync.dma_start(wt[:], d['wo_w'][li])
                for m in range(NDC):
                    ps = ps_mm.tile([128, NB], F32, tag="mm")
                    for k in range(NDC):
                        nc.tensor.matmul(ps[:], wt[:, m, k, :], oL[:, k, :],
                                         start=(k == 0), stop=(k == NDC - 1))
                    st = sq_p.tile([128, NB], F32R, tag="sqL")
                    nc.scalar.activation(st[:], ps[:], IDENT, bias=bL[:, BO + m:BO + m + 1])
                    nc.vector.tensor_tensor(out=hL[:, m, :], in0=hL[:, m, :],
                                            in1=st[:], op=AOP.add)
            else:
                # ---- Wo + residual
                wt = wp.tile([128, NDC, NDC, 128], BF16, tag="wp")
                nc.sync.dma_start(wt[:], d['wo_w'][li])
                for hf in range(2):
                    for m in range(NDC):
                        ps = ps_mm.tile([128, HT], F32, tag="mm")
                        for k in range(NDC):
                            nc.tensor.matmul(ps[:], wt[:, m, k, :],
                                             oT[:, k, hf * HT:(hf + 1) * HT],
                                             start=(k == 0), stop=(k == NDC - 1))
                        st = sq_p.tile([128, HT], F32R, tag="sq")
                        nc.scalar.activation(st[:], ps[:], IDENT, bias=bL[:, BO + m:BO + m + 1])
                        nc.vector.tensor_tensor(out=hT[:, m, hf * HT:(hf + 1) * HT],
                                                in0=hT[:, m, hf * HT:(hf + 1) * HT],
                                                in1=st[:], op=AOP.add)
            # ---- FFN
            if not last:
                aT2 = sb1.tile([128, NDC, T], BF16, tag="aT")
                for hf in range(2):
                    ln_half(hT, hf * HT, HT, aT2, hf * HT)
                ffq3 = sb1.tile([128, NDC, T], BF16, tag="tagF")
                ffqs = [qT, kT, oT, ffq3]
                for g in range(4):
                    wt = wp.tile([128, 8, NDC, 128], BF16, tag="wp")
                    nc.sync.dma_start(wt[:], d['w1_w'][li, g])
                    for hf in range(2):
                        for m8 in range(8):
                            m = g * 8 + m8
                            ps = ps_mm.tile([128, HT], F32, tag="mm")
                            for k in range(NDC):
                                nc.tensor.matmul(ps[:], wt[:, m8, k, :],
                                                 aT2[:, k, hf * HT:(hf + 1) * HT],
                                                 start=(k == 0), stop=(k == NDC - 1))
                            nc.scalar.activation(ffqs[g][:, m8, hf * HT:(hf + 1) * HT],
                                                 ps[:], RELU, bias=bL[:, B1 + m:B1 + m + 1])
                for g in range(4):
                    wt = wp.tile([128, 2, NFC, 128], BF16, tag="wp")
                    nc.sync.dma_start(wt[:], d['w2_w'][li, g])
                    for hf in range(2):
                        for m2 in range(2):
                            m = g * 2 + m2
                            ps = ps_mm.tile([128, HT], F32, tag="mm")
                            for k in range(NFC):
                                nc.tensor.matmul(ps[:], wt[:, m2, k, :],
                                                 ffqs[k // 8][:, k % 8, hf * HT:(hf + 1) * HT],
                                                 start=(k == 0), stop=(k == NFC - 1))
                            st = sq_p.tile([128, HT], F32R, tag="sq")
                            nc.scalar.activation(st[:], ps[:], IDENT,
                                                 bias=bL[:, B2 + m:B2 + m + 1])
                            nc.vector.tensor_tensor(out=hT[:, m, hf * HT:(hf + 1) * HT],
                                                    in0=hT[:, m, hf * HT:(hf + 1) * HT],
                                                    in1=st[:], op=AOP.add)
            else:
                # last layer: FFN only for the last token of each batch
                # (hL already holds the post-attention residual for those tokens)
                aL = sb1.tile([128, NDC, NB], BF16, tag="aL")
                ln_half(hL, 0, NB, aL, 0)
                ffL = sb1.tile([128, NFC, NB], BF16, tag="ffL")
                for g in range(4):
                    wt = wp.tile([128, 8, NDC, 128], BF16, tag="wp")
                    nc.sync.dma_start(wt[:], d['w1_w'][li, g])
                    for m8 in range(8):
                        m = g * 8 + m8
                        ps = ps_mm.tile([128, NB], F32, tag="mm")
                        for k in range(NDC):
                            nc.tensor.matmul(ps[:], wt[:, m8, k, :], aL[:, k, :],
                                             start=(k == 0), stop=(k == NDC - 1))
                        nc.scalar.activation(ffL[:, m, :], ps[:], RELU,
                                             bias=bL[:, B1 + m:B1 + m + 1])
                for g in range(4):
                    wt = wp.tile([128, 2, NFC, 128], BF16, tag="wp")
                    nc.sync.dma_start(wt[:], d['w2_w'][li, g])
                    for m2 in range(2):
                        m = g * 2 + m2
                        ps = ps_mm.tile([128, NB], F32, tag="mm")
                        for k in range(NFC):
                            nc.tensor.matmul(ps[:], wt[:, m2, k, :], ffL[:, k, :],
                                             start=(k == 0), stop=(k == NFC - 1))
                        st = sq_p.tile([128, NB], F32R, tag="sqL")
                        nc.scalar.activation(st[:], ps[:], IDENT,
                                             bias=bL[:, B2 + m:B2 + m + 1])
                        nc.vector.tensor_tensor(out=hL[:, m, :], in0=hL[:, m, :],
                                                in1=st[:], op=AOP.add)

        # ---------------- head ----------------
        if hL is None:
            # debug path (fewer layers): extract last tokens now
            hL = sb1.tile([128, NDC, NB], F32R, tag="hL")
            for c in range(NDC):
                nc.vector.tensor_copy(
                    hL[:, c, :],
                    hT[:, c, :].rearrange("p (b s) -> p b s", s=S)[:, :, S - 1])
        pL = sb1.tile([128, NDC, NB], BF16, tag="pL")
        ln_half(hL, 0, NB, pL, 0)
        cw = wp.tile([128, NDC, NDC, 128], BF16, tag="wp")
        nc.sync.dma_start(cw[:], d['cf_w'][:])
        cb = brow_p.tile([128, NDC], F32, tag="brow")
        nc.sync.dma_start(cb[:], d['cf_bT'][:])
        z1 = sb1.tile([128, NDC, NB], BF16, tag="z1")
        for m in range(NDC):
            ps = ps_mm.tile([128, NB], F32, tag="mm")
            for k in range(NDC):
                nc.tensor.matmul(ps[:], cw[:, m, k, :], pL[:, k, :],
                                 start=(k == 0), stop=(k == NDC - 1))
            nc.scalar.activation(z1[:, m, :], ps[:], RELU, bias=cb[:, m:m + 1])
        fwt = sb1.tile([128, NDC, NCLS], BF16, tag="fwt")
        nc.sync.dma_start(fwt[:], d['fc_w'][:])
        fb = brow_p.tile([NCLS, 1], F32, tag="brow2")
        nc.sync.dma_start(fb[:], d['fc_b'][:])
        ps = ps_mm.tile([NCLS, NB], F32, tag="mm")
        for k in range(NDC):
            nc.tensor.matmul(ps[:], fwt[:, k, :], z1[:, k, :],
                             start=(k == 0), stop=(k == NDC - 1))
        osb = sb1.tile([NCLS, NB], F32, tag="osb")
        nc.vector.tensor_scalar_add(osb[:], ps[:], fb[:])
        nc.sync.dma_start(out[:], osb[:])


def _prep_weights(inputs, n_layers=L):
    from ml_dtypes import bfloat16
    f64 = np.float64

    def lhsT_pack(W, nk, nm):
        # W [K, M] -> [128, nm, nk, 128]: out[p, m, k, c] = W[k*128+p, m*128+c]
        return np.ascontiguousarray(
            W.reshape(nk, 128, nm, 128).transpose(1, 2, 0, 3))

    g = {}
    emb = inputs['embed_w'].astype(f64)          # [1200, 1024]
    pos = np.arange(S, dtype=f64)[:, None]
    div = np.exp(np.arange(0, D, 2, dtype=np.float32).astype(f64) * (-math.log(10000.0) / D))
    pe = np.zeros((S, D), f64)
    pe[:, 0::2] = np.sin(pos * div)
    pe[:, 1::2] = np.cos(pos * div)
    Wp = np.zeros((NIP, D), f64)
    Wp[:NI] = emb
    Wp[NI:NI + S] = pe
    arr = lhsT_pack(Wp, NKI, NDC)                # [128, 8, 10, 128]
    g['emb_w'] = np.ascontiguousarray(
        arr.reshape(128, 2, 4, NKI, 128).transpose(1, 0, 2, 3, 4)).astype(bfloat16)

    ln_g = inputs['ln_g'].astype(f64); ln_b = inputs['ln_b'].astype(f64)
    aw = inputs['attn_w'].astype(f64); ab = inputs['attn_b'].astype(f64)
    fw1 = inputs['ff_w1'].astype(f64); fb1 = inputs['ff_b1'].astype(f64)
    fw2 = inputs['ff_w2'].astype(f64); fb2 = inputs['ff_b2'].astype(f64)

    qk_w = np.zeros((L, 2, 128, NDC, NDC, 128), bfloat16)
    wv_w = np.zeros((L, 128, NDC, D), bfloat16)
    wo_w = np.zeros((L, 128, NDC, NDC, 128), bfloat16)
    w1_w = np.zeros((L, 4, 128, 8, NDC, 128), bfloat16)
    w2_w = np.zeros((L, 4, 128, 2, NFC, 128), bfloat16)
    bLa = np.zeros((L, 128, 64), np.float32)

    for i in range(n_layers):
        g1, b1 = ln_g[i, 0][:, None], ln_b[i, 0]
        for mat in range(2):
            We = g1 * aw[i, mat]
            be = ab[i, mat] + b1 @ aw[i, mat]
            qk_w[i, mat] = lhsT_pack(We, NDC, NDC).astype(bfloat16)
            bLa[i, :, mat * NDC:(mat + 1) * NDC] = be.reshape(NDC, 128).T
        # V (natural layout rhs); bias folded into wo_b (softmax rows sum to 1)
        Wve = g1 * aw[i, 2]
        bv = ab[i, 2] + b1 @ aw[i, 2]
        wv_w[i] = np.ascontiguousarray(
            Wve.reshape(NDC, 128, D).transpose(1, 0, 2)).astype(bfloat16)
        wo_w[i] = lhsT_pack(aw[i, 3], NDC, NDC).astype(bfloat16)
        wo_be = ab[i, 3] + bv @ aw[i, 3]
        bLa[i, :, BO:BO + NDC] = wo_be.reshape(NDC, 128).T
        g2, b2 = ln_g[i, 1][:, None], ln_b[i, 1]
        W1e = g2 * fw1[i]
        b1e = fb1[i] + b2 @ fw1[i]
        arr = lhsT_pack(W1e, NDC, NFC)           # [128, 32, 8, 128]
        w1_w[i] = arr.reshape(128, 4, 8, NDC, 128).transpose(1, 0, 2, 3, 4).astype(bfloat16)
        bLa[i, :, B1:B1 + NFC] = b1e.reshape(NFC, 128).T
        arr = lhsT_pack(fw2[i], NFC, NDC)        # [128, 8, 32, 128]
        w2_w[i] = arr.reshape(128, 4, 2, NFC, 128).transpose(1, 0, 2, 3, 4).astype(bfloat16)
        bLa[i, :, B2:B2 + NDC] = fb2[i].reshape(NDC, 128).T

    g['qk_w'] = qk_w; g['wv_w'] = wv_w; g['wo_w'] = wo_w
    g['w1_w'] = w1_w; g['w2_w'] = w2_w; g['bL'] = bLa

    inv = 1.0 / math.sqrt(1.0 + 1e-5)
    fin_g = inputs['fin_g'].astype(f64); fin_b = inputs['fin_b'].astype(f64)
    A1 = fin_g * inv * inputs['cf_bn_g'].astype(f64)
    C1 = fin_b * inv * inputs['cf_bn_g'].astype(f64) + inputs['cf_bn_b'].astype(f64)
    cfw = inputs['cf_w'].astype(f64)
    cf_we = A1[:, None] * cfw
    cf_be = inputs['cf_b'].astype(f64) + C1 @ cfw
    g['cf_w'] = lhsT_pack(cf_we, NDC, NDC).astype(bfloat16)
    g['cf_bT'] = cf_be.reshape(NDC, 128).T.astype(np.float32)
    A2 = inv * inputs['fc_bn_g'].astype(f64)
    C2 = inputs['fc_bn_b'].astype(f64)
    fcw = inputs['fc_w'].astype(f64)
    fc_we = A2[:, None] * fcw
    fc_be = inputs['fc_b'].astype(f64) + C2 @ fcw
    g['fc_w'] = np.ascontiguousarray(
        fc_we.reshape(NDC, 128, NCLS).transpose(1, 0, 2)).astype(bfloat16)
    g['fc_b'] = fc_be.reshape(NCLS, 1).astype(np.float32)
    g['ones'] = np.ones((128, 512), np.float32)
    return g


def _run_timed(nc, in_maps, n_iters=10):
    """Mirror bass2jax.run_bass_via_pjrt (no donation), time steady-state execs."""
    import time
    import jax
    import numpy as _np
    from jax.experimental.shard_map import shard_map
    from jax.sharding import Mesh, PartitionSpec, NamedSharding
    from concourse import bass2jax as b2j
    from concourse import mybir as _mb

    b2j.install_neuronx_cc_hook()
    n_cores = len(in_maps)
    partition_name = nc.partition_id_tensor.name if nc.partition_id_tensor else None
    in_names, out_names, out_avals, zero_outs = [], [], [], []
    for alloc in nc.m.functions[0].allocations:
        if not isinstance(alloc, _mb.MemoryLocationSet):
            continue
        name = alloc.memorylocations[0].name
        if alloc.kind == "ExternalInput":
            if name != partition_name:
                in_names.append(name)
        elif alloc.kind == "ExternalOutput":
            shape = tuple(alloc.tensor_shape)
            dtype = _mb.dt.np(alloc.dtype)
            out_names.append(name)
            out_avals.append(jax.core.ShapedArray(shape, dtype))
            zero_outs.append(_np.zeros(shape, dtype))
    n_params = len(in_names)
    all_in_names = list(in_names) + list(out_names)
    if partition_name is not None:
        all_in_names.append(partition_name)

    def _body(*args):
        operands = list(args)
        if partition_name is not None:
            operands.append(b2j.partition_id_tensor())
        outs = b2j._bass_exec_p.bind(
            *operands,
            out_avals=tuple(out_avals),
            in_names=tuple(all_in_names),
            out_names=tuple(out_names),
            lowering_input_output_aliases=(),
            sim_require_finite=True,
            sim_require_nnan=True,
            nc=nc,
        )
        return tuple(outs)

    devices = jax.devices()[:n_cores]
    mesh = Mesh(_np.asarray(devices), ("core",))
    spec = PartitionSpec("core")
    sharded = jax.jit(shard_map(
        _body, mesh=mesh, in_specs=(spec,) * (n_params + len(out_names)),
        out_specs=(spec,) * len(out_names), check_rep=False))
    sh = NamedSharding(mesh, spec)
    concat_in = [
        jax.device_put(_np.concatenate([_np.asarray(m[name]) for m in in_maps], axis=0), sh)
        for name in in_names
    ]
    concat_zeros = [
        jax.device_put(_np.zeros((n_cores * z.shape[0], *z.shape[1:]), z.dtype), sh)
        for z in zero_outs
    ]
    outs = sharded(*concat_in, *concat_zeros)
    jax.block_until_ready(outs)
    t0 = time.time()
    for _ in range(n_iters):
        outs = sharded(*concat_in, *concat_zeros)
    jax.block_until_ready(outs)
    t1 = time.time()
    per_call_ns = (t1 - t0) / n_iters * 1e9
    results = [
        {name: _np.asarray(outs[i]).reshape(n_cores, *out_avals[i].shape)[c]
         for i, name in enumerate(out_names)}
        for c in range(n_cores)
    ]
    return results, per_call_ns


def kernel(**inputs):
    global LAST_EXEC_NS
    from ml_dtypes import bfloat16
    n_layers = int(inputs.pop('_n_layers', L))
    if n_layers not in _CACHE:
        _CACHE[n_layers] = _build(n_layers)
    nc = _CACHE[n_layers]
    g = _prep_weights(inputs, n_layers)

    x = inputs['x']
    xr = np.asarray(x).reshape(B, S, NI)
    in_maps = []
    for ci in range(NCORES):
        xc = xr[ci * NB:(ci + 1) * NB].astype(np.float64)  # [16, 50, 1200]
        xa = np.zeros((NB, S, NIP), np.float32)
        xa[:, :, :NI] = xc
        xa[np.arange(NB)[:, None], np.arange(S)[None, :], NI + np.arange(S)[None, :]] = 1.0
        # xT [128, NKI*T]: out[p, k*T + t] = xa_feat[k*128+p, t]
        xf = xa.reshape(T, NIP).T.reshape(NKI, 128, T).transpose(1, 0, 2)
        m = dict(g)
        m['xT'] = np.ascontiguousarray(xf.reshape(128, NKI * T)).astype(bfloat16)
        in_maps.append(m)

    if TRACE:
        results, per_call_ns = _run_timed(nc, in_maps)
        LAST_EXEC_NS = int(per_call_ns)
    else:
        res = run_bass_kernel_spmd(nc, in_maps, core_ids=list(range(NCORES)))
        LAST_EXEC_NS = res.exec_time_ns
        results = res.results
    outs = [r['out'].T for r in results]   # each [NB, NCLS]
    return np.concatenate(outs, axis=0).astype(np.float32)
